# revision 32
# baseline (speedup 1.0000x reference)
"""Trainium2 Bass kernel for the 2-layer grid-GCN + linear head.

Math: the GCN aggregation over the fixed 26x26 grid is a banded linear
operator on the node axis (halfwidth 26): per batch column

    h1 = relu(B1 @ xT + b1)      B1 = w1 * A
    v2 = B2s @ h1                B2s = diag(lw) * w2 * A   (head folded in)
    y  = relu(sum_j lw_j relu(v2_j / lw_j) + lin_b)

Since psum_j = lw_j * v2_j, the head contribution is
    c_j = lw_j * relu(psum_j / lw_j) = max(min(psum_j, hi_j), lo_j)
with hi_j = +BIG if lw_j>0 else 0, lo_j = -BIG if lw_j<0 else 0: a single
DVE tensor_scalar (min,max) with per-partition bounds.  y = relu(ones^T
(sum_k c_k) + lin_b) via a ones-matmul partition reduce.

Engine budget per 1024-col block (8 blocks/core, batch sharded 8 ways):
  PE     44 matmuls x 512 cols  ~9.5 us  (bound)
  Act    6 conv1 relu drains + 1 head relu  ~8.3 us
  DVE    6 conv2 min/max drains + 2 adds    ~8.8 us
  GpSimd 3 accumulator adds                 ~6.8 us
PSUM: one tag, 4 bufs x [128,1024] f32 = all 8 banks (depth-4 rotation).
Head matmuls of block b are emitted inside block b+1 so the in-order PE
queue never waits on the DVE/GpSimd add tree.  All weights arrive in one
packed DMA; x tiles are [<=128, 2048] groups, startup split across two
DMA queues so block 0's tail stages aren't transfer-bound.
"""

import sys

if "/opt/trn_rl_repo" not in sys.path:
    sys.path.insert(0, "/opt/trn_rl_repo")

import numpy as np
import ml_dtypes

N_CORES = 8
N = 676                        # nodes (26x26 grid)
B_TOTAL = 65536
COLS = B_TOTAL // N_CORES      # batch columns per core (8192)
CHUNK = 512                    # matmul moving free dim / PSUM bank
BLOCK = 1024                   # processing block (2 chunks)
GROUP = 2048                   # x DMA column-group (2 blocks)
N_BLOCKS = COLS // BLOCK       # 8
N_GROUPS = COLS // GROUP       # 4
HALO = 52                      # 2 * band halfwidth

XLO = [0, 76, 204, 332, 460, 548]         # x tile starts (tiles 0/5 widened)
XHI = [128, 204, 332, 460, 588, 676]      # x tile ends
HS = [0, 102, 230, 358, 486, 614, 676]    # h1 tiling
ZS = [0, 128, 256, 384, 512, 640, 676]    # h2 tiling
NT = 6

PX = [XHI[k] - XLO[k] for k in range(NT)]
PH = [HS[k + 1] - HS[k] for k in range(NT)]
PZ = [ZS[k + 1] - ZS[k] for k in range(NT)]

# All matmuls run with K=128: small-K matmuls stream ~50% slower on the
# PE, so halo stationaries are padded to 128 rows (entries outside the
# band are zero; overlap rows are zeroed by hand) and the h1_0/h1_5
# moving tiles are padded to 128 partitions (pad rows memset once, their
# stationary rows are zero).
K1P = {k: 128 for k in range(1, NT - 1)}   # conv1 halo contraction depth
K2P = {k: 128 for k in range(NT - 1)}      # conv2 halo contraction depth
N_BURN = 12              # HAM ignition matmuls at startup

bf16 = ml_dtypes.bfloat16
BIG = 3.0e38

TRACE = False            # test.py flips this to profile
LAST_RESULT = None       # BassKernelResults stash when TRACE

_PROGRAM_CACHE = {}


def _wpk_layout():
    """Column offsets of each weight block in the packed bf16 tensor."""
    off = {}
    c = 0
    for k in range(NT):
        off[f"w1m{k}"] = c
        c += PH[k]
    for k in range(1, NT - 1):
        off[f"w1h{k}"] = c
        c += PH[k]
    for k in range(NT):
        off[f"w2m{k}"] = c
        c += 128
    for k in range(NT - 1):
        off[f"w2h{k}"] = c
        c += 128
    off["ones"] = c
    c += 1
    return off, c


def _build_program(b1f, b2f, linbf):
    key = (b1f, b2f, linbf)
    if key in _PROGRAM_CACHE:
        return _PROGRAM_CACHE[key]

    import concourse.mybir as mybir
    import concourse.tile as tile
    from concourse import bacc

    nc = bacc.Bacc(None, target_bir_lowering=False)
    dt = mybir.dt
    relu = mybir.ActivationFunctionType.Relu
    Alu = mybir.AluOpType

    woff, WCOLS = _wpk_layout()

    xt_d = nc.dram_tensor("xt", (N, COLS), dt.bfloat16, kind="ExternalInput")
    wpk_d = nc.dram_tensor("wpk", (128, WCOLS), dt.bfloat16,
                           kind="ExternalInput")
    # f32 pack: cols 0-5 hi, 6-11 lo, 12-17 sgn, 18-23 wb2
    hilo_d = nc.dram_tensor("hilo", (128, 24), dt.float32,
                            kind="ExternalInput")
    zpad_d = nc.dram_tensor("zpad", (66, BLOCK), dt.bfloat16,
                            kind="ExternalInput")
    y_d = nc.dram_tensor("y", (1, COLS), dt.float32, kind="ExternalOutput")

    with tile.TileContext(nc) as tc:
        with (
            tc.tile_pool(name="weights", bufs=1) as wpool,
            tc.tile_pool(name="xin", bufs=2) as xpool,
            tc.tile_pool(name="h1", bufs=2) as hpool,
            tc.tile_pool(name="cacc", bufs=2) as cpool,
            tc.tile_pool(name="yout", bufs=1) as ypool,
            tc.tile_pool(name="ps", bufs=4, space="PSUM") as pspool,
        ):
            wpk = wpool.tile([128, WCOLS], dt.bfloat16, tag="wpk")
            hilo = wpool.tile([128, 24], dt.float32, tag="hilo")

            def wap(name, p, w):
                o = woff[name]
                return wpk[0:p, o:o + w]

            w1m = [wap(f"w1m{k}", PX[k], PH[k]) for k in range(NT)]
            w1h = {k: wap(f"w1h{k}", K1P[k], PH[k]) for k in range(1, NT - 1)}
            w2m = [wap(f"w2m{k}", 128, 128) for k in range(NT)]
            w2h = [wap(f"w2h{k}", 128, 128) for k in range(NT - 1)]
            ones = wap("ones", 128, 1)

            y_sb = ypool.tile([1, COLS], dt.float32, tag="y")

            # x tiles: xt_t[k][g], [PX[k], GROUP] bf16
            xt_t = [[None] * N_GROUPS for _ in range(NT)]

            def dma_x_tile(k, g, eng):
                t = xpool.tile([PX[k], GROUP], dt.bfloat16,
                               tag=f"x{k}", name=f"x{k}_{g}")
                xt_t[k][g] = t
                c0 = g * GROUP
                eng.dma_start(t[:], xt_d[XLO[k]:XHI[k], c0:c0 + GROUP])

            # HAM ignition: a dense burst of dummy matmuls while the x DMAs
            # land, so the PE un-throttles to 2.4 GHz before real work.
            burn = wpool.tile([128, CHUNK], dt.bfloat16, tag="burn")
            nc.gpsimd.memset(burn[:], 0.0)
            for i in range(N_BURN):
                bp = pspool.tile([128, CHUNK], dt.float32, tag="ps",
                                 name=f"burn_{i}")
                nc.tensor.matmul(bp[:], burn[0:128, 0:128], burn[:],
                                 start=True, stop=True)

            # Startup DMAs: packed weights on the scalar queue (idle until
            # the first drain), x even tiles on sync, odd on gpsimd, so
            # x0..x5 land before block 0's stages need them.
            nc.scalar.dma_start(wpk[:], wpk_d[:])
            dma_x_tile(0, 0, nc.sync)
            dma_x_tile(1, 0, nc.gpsimd)
            dma_x_tile(2, 0, nc.sync)
            dma_x_tile(3, 0, nc.gpsimd)
            dma_x_tile(4, 0, nc.sync)
            nc.sync.dma_start(hilo[:], hilo_d[:])
            dma_x_tile(5, 0, nc.gpsimd)
            # group 1 is triggered from inside block 0 (scalar/gpsimd
            # queues) so its transfers don't crowd out wpk/group-0 on HBM
            # during the startup window

            # per-block state for the deferred head phase
            acc_t = [None] * N_BLOCKS

            def head_phase(b):
                """Partition-reduce acc(b) to y[b*BLOCK:...], then DMA out."""
                psl = pspool.tile([1, BLOCK], dt.float32, tag="ps",
                                  name=f"psl_{b}")
                for c in range(2):
                    cc = slice(c * CHUNK, (c + 1) * CHUNK)
                    nc.tensor.matmul(psl[0:1, cc], ones, acc_t[b][:, cc],
                                     start=True, stop=True)
                col = b * BLOCK
                nc.scalar.activation(y_sb[0:1, col:col + BLOCK],
                                     psl[0:1, 0:BLOCK], relu, bias=linbf)
                nc.sync.dma_start(y_d[0:1, col:col + BLOCK],
                                  y_sb[0:1, col:col + BLOCK])

            for b in range(N_BLOCKS):
                g, h = divmod(b, 2)
                bc = slice(h * BLOCK, (h + 1) * BLOCK)  # cols within group

                if h == 0 and g >= 1 and g + 1 < N_GROUPS:
                    for k in range(NT):
                        dma_x_tile(k, g + 1, nc.sync)

                h1_t = [None] * NT

                def conv1_stage(k):
                    has_halo = 1 <= k < NT - 1
                    ps = pspool.tile([PH[k], BLOCK], dt.float32, tag="ps",
                                     name=f"ps1_{k}_{b}")
                    for c in range(2):
                        cc = slice(h * BLOCK + c * CHUNK,
                                   h * BLOCK + (c + 1) * CHUNK)
                        pc = slice(c * CHUNK, (c + 1) * CHUNK)
                        nc.tensor.matmul(ps[:, pc], w1m[k],
                                         xt_t[k][g][:, cc],
                                         start=True, stop=not has_halo)
                    if has_halo:
                        for c in range(2):
                            cc = slice(h * BLOCK + c * CHUNK,
                                       h * BLOCK + (c + 1) * CHUNK)
                            pc = slice(c * CHUNK, (c + 1) * CHUNK)
                            nc.tensor.matmul(ps[:, pc], w1h[k],
                                             xt_t[k + 1][g][0:K1P[k], cc],
                                             start=False, stop=True)
                    h1_t[k] = hpool.tile([128, BLOCK], dt.bfloat16,
                                         tag=f"h1_{k}", name=f"h1_{k}_{b}")
                    nc.scalar.activation(h1_t[k][0:PH[k], :], ps[:], relu,
                                         bias=b1f)
                    if b < 2 and PH[k] < 128:
                        # zero the pad rows once per buffer via a tiny DMA
                        # (their conv2 stationary rows are zero; NaN*0
                        # must not occur; scalar HWDGE queue is idle)
                        nc.scalar.dma_start(h1_t[k][PH[k]:128, :],
                                            zpad_d[0:128 - PH[k], :])

                c_t = [None] * NT

                def conv2_stage(k):
                    last = k == NT - 1
                    ps = pspool.tile([128, BLOCK], dt.float32, tag="ps",
                                     name=f"ps2_{k}_{b}")
                    for c in range(2):
                        pc = slice(c * CHUNK, (c + 1) * CHUNK)
                        nc.tensor.matmul(ps[:, pc], w2m[k],
                                         h1_t[k][0:128, pc],
                                         start=True, stop=last)
                    if not last:
                        for c in range(2):
                            pc = slice(c * CHUNK, (c + 1) * CHUNK)
                            nc.tensor.matmul(ps[:, pc], w2h[k],
                                             h1_t[k + 1][0:K2P[k], pc],
                                             start=False, stop=True)
                    c_t[k] = cpool.tile([128, BLOCK], dt.bfloat16,
                                        tag=f"c{k}", name=f"c{k}_{b}")
                    if b2f == 0.0 and k == 0:
                        # offload one stage's PSUM drain to Act:
                        # u = relu(sgn*ps) = |c|, then a cheap 4x-mode
                        # SBUF tensor_scalar applies the sign on DVE
                        u = cpool.tile([128, BLOCK], dt.bfloat16,
                                       tag=f"u{k}", name=f"u{k}_{b}")
                        nc.scalar.activation(
                            u[:], ps[:], relu,
                            scale=hilo[:, 12 + k:13 + k])
                        nc.vector.tensor_scalar(
                            c_t[k][:], u[:], hilo[:, 12 + k:13 + k], None,
                            Alu.mult)
                    elif b2f == 0.0:
                        # c = max(min(ps, hi), lo) = lw * relu(v2)
                        nc.vector.tensor_scalar(
                            c_t[k][:], ps[:],
                            hilo[:, k:k + 1], hilo[:, 6 + k:7 + k],
                            Alu.min, Alu.max)
                    else:
                        # u = relu(sgn*ps + |lw|*b2); c = sgn*u
                        u = cpool.tile([128, BLOCK], dt.bfloat16,
                                       tag=f"u{k}", name=f"u{k}_{b}")
                        nc.scalar.activation(
                            u[:], ps[:], relu,
                            bias=hilo[:, 18 + k:19 + k],
                            scale=hilo[:, 12 + k:13 + k])
                        nc.vector.tensor_scalar(
                            c_t[k][:], u[:], hilo[:, 12 + k:13 + k], None,
                            Alu.mult)

                def tadd(eng, tag, a, bb):
                    t = cpool.tile([128, BLOCK], dt.bfloat16, tag=tag,
                                   name=f"{tag}_{b}")
                    eng.tensor_tensor(t[:], a[:], bb[:], Alu.add)
                    return t

                # staircase; head of block b-1 rides mid block b so the PE
                # has ~18 matmuls of cover while b-1's add tree finishes
                conv1_stage(0)
                if b == 0:
                    for k in range(0, NT, 2):
                        dma_x_tile(k, 1, nc.scalar)
                conv1_stage(1)
                conv2_stage(0)
                conv1_stage(2)
                if b == 0:
                    for k in range(1, NT, 2):
                        dma_x_tile(k, 1, nc.gpsimd)
                conv2_stage(1)
                s1 = tadd(nc.gpsimd, "s1", c_t[0], c_t[1])
                conv1_stage(3)
                if b >= 2:
                    head_phase(b - 1)
                conv2_stage(2)
                conv1_stage(4)
                if b == 1:
                    # block 0's add tree straggles (cold PE, DMA waits):
                    # give its head more cover
                    head_phase(0)
                conv1_stage(5)
                conv2_stage(3)
                s2 = tadd(nc.gpsimd, "s2", c_t[2], c_t[3])
                # s12 on DVE: GpSimd's 2.1us adds land too late and stall
                # the deferred head via acc
                s12 = tadd(nc.vector, "s12", s1, s2)
                conv2_stage(4)
                conv2_stage(5)
                s3 = tadd(nc.vector, "s3", c_t[4], c_t[5])
                acc_t[b] = tadd(nc.vector, "acc", s12, s3)

            head_phase(N_BLOCKS - 1)

    nc.compile()
    _PROGRAM_CACHE[key] = nc
    return nc


def kernel(x, w1, b1, w2, b2, lin_w, lin_b, edge_src, edge_dst):
    global LAST_RESULT
    from concourse import bass_utils

    x = np.asarray(x)
    # Dense normalized aggregation operator from the edge lists.
    deg = np.zeros(N, np.float64)
    np.add.at(deg, np.asarray(edge_dst), 1.0)
    dinv = 1.0 / np.sqrt(deg)
    normv = dinv[np.asarray(edge_src)] * dinv[np.asarray(edge_dst)]
    A = np.zeros((N, N), np.float64)
    np.add.at(A, (np.asarray(edge_dst), np.asarray(edge_src)), normv)

    w1f = float(np.asarray(w1).reshape(-1)[0])
    w2f = float(np.asarray(w2).reshape(-1)[0])
    b1f = float(np.asarray(b1).reshape(-1)[0])
    b2f = float(np.asarray(b2).reshape(-1)[0])
    linbf = float(np.asarray(lin_b).reshape(-1)[0])
    lw = np.asarray(lin_w).reshape(-1).astype(np.float64)

    B1 = w1f * A
    B2s = (lw[:, None]) * (w2f * A)   # head weights folded into conv2 rows

    woff, WCOLS = _wpk_layout()
    wpk = np.zeros((128, WCOLS), np.float64)

    def put(name, block):
        p, w = block.shape
        wpk[0:p, woff[name]:woff[name] + w] = block

    for k in range(NT):
        put(f"w1m{k}", B1[HS[k]:HS[k + 1], XLO[k]:XHI[k]].T)
        if 1 <= k < NT - 2:
            # K padded to 128 rows; entries beyond the 52-halo are 0 (band)
            put(f"w1h{k}", B1[HS[k]:HS[k + 1], XHI[k]:XHI[k] + K1P[k]].T)
        elif k == NT - 2:
            # x_5 tile starts at 548 (widened): zero the 40 overlap rows
            # already covered by the main matmul, keep [588, 676).
            blk = np.zeros((128, PH[k]), np.float64)
            blk[XHI[k] - XLO[k + 1]:, :] = B1[HS[k]:HS[k + 1],
                                              XHI[k]:676].T
            put(f"w1h{k}", blk)
        m2 = np.zeros((128, 128), np.float64)
        m2[0:PH[k], 0:PZ[k]] = B2s[ZS[k]:ZS[k + 1], HS[k]:HS[k + 1]].T
        put(f"w2m{k}", m2)
        if k < NT - 2:
            h2b = np.zeros((128, 128), np.float64)
            h2b[:, 0:PZ[k]] = B2s[ZS[k]:ZS[k + 1],
                                  HS[k + 1]:HS[k + 1] + 128].T
            put(f"w2h{k}", h2b)
        elif k == NT - 2:
            h2b = np.zeros((128, 128), np.float64)
            h2b[0:PH[k + 1], 0:PZ[k]] = B2s[ZS[k]:ZS[k + 1],
                                            HS[k + 1]:676].T
            put(f"w2h{k}", h2b)
    put("ones", np.ones((128, 1)))

    hilo = np.zeros((128, 24), np.float32)
    for k in range(NT):
        lwk = lw[ZS[k]:ZS[k + 1]]
        hilo[0:PZ[k], k] = np.where(lwk > 0, BIG, 0.0)
        hilo[0:PZ[k], 6 + k] = np.where(lwk < 0, -BIG, 0.0)
        hilo[0:PZ[k], 12 + k] = np.sign(lwk)
        hilo[0:PZ[k], 18 + k] = np.abs(lwk) * b2f

    in_map = {
        "wpk": wpk.astype(bf16),
        "hilo": hilo,
        "zpad": np.zeros((66, BLOCK), dtype=bf16),
    }

    nc = _build_program(b1f, b2f, linbf)

    # host-side: transpose, cast, shard along batch
    xt = np.ascontiguousarray(x.T).astype(bf16)        # [676, 65536]
    in_maps = []
    for c in range(N_CORES):
        m = dict(in_map)
        m["xt"] = np.ascontiguousarray(xt[:, c * COLS:(c + 1) * COLS])
        in_maps.append(m)

    res = bass_utils.run_bass_kernel_spmd(
        nc, in_maps, list(range(N_CORES)), trace=TRACE
    )
    if TRACE:
        LAST_RESULT = res
    out = np.concatenate([res.results[c]["y"].reshape(-1) for c in range(N_CORES)])
    return out.reshape(B_TOTAL, 1).astype(np.float32)
